# revision 11
# baseline (speedup 1.0000x reference)
"""A-trous wavelet decomposition (db2, J=8) Trainium2 Bass kernel.

Math: each scale-j filter h_j (length 3*2^j + 1) is the cumsum of the
dilated db2 dec_lo, i.e. piecewise constant with 4 segments:
  h_j = [c0]*s + [c1]*s + [c2]*s + [c3],  s = 2^j,  c_i = cumsum(dec_lo)_i * 2^(-j/2)
So conv(x, h_j) = c0*S_j[t-3s/2] + c1*S_j[t-s/2] + c2*S_j[t+s/2] + c3*x[t+3s/2]
with S_j = sliding window sum of length 2^j, built by recursive doubling:
  S_k[i] = S_{k-1}[i] + S_{k-1}[i + 2^(k-1)]

Sharding: pure data parallel over batch (256 = 8 cores x 32 rows).
Per-core layout: 128 partitions = 4 sequence chunks x 32 rows, each
partition holds a 4096-column chunk plus 384-column halos (max tap reach
at j=8 is +-384). The halo-replicated [128, 4864] layout is prepared on
the host so the device input is a single contiguous DMA.

Engine balance (cols of each 4096-wide output row):
  - left [0, CS): Vector engine runs the fused Horner chain
    (scalar_tensor_tensor: out = in0*r + in1), Scalar engine applies the
    final scale multiply.
  - right [CS, 4096): Scalar engine computes the 4 scaled taps, GPSIMD
    pair-adds them.
  - window-sum pyramid: Vector engine.
This splits the add-bound work across DVE+GPSIMD and the multiply work
onto ACT, with all three engines ~equally busy.

Walrus codegen only has 1-2 sync-wait slots per compute-instruction
encoding, so a post-pass moves excess Tile-generated waits onto
standalone EventSemaphore instructions.
"""

import numpy as np

B, S = 256, 16384
NCORES = 8
RPC = B // NCORES          # rows per core = 32
J = 8
NS = J + 1                 # 9 scales
OLEN = S - 1               # 16383 output length
H = 3 * (1 << (J - 1))     # 384 halo
O = 4096                   # output columns per partition-chunk
W = O + 2 * H              # 4864 columns per partition incl. halos
CS = 2184                  # DVE/(ACT+GPSIMD) column split of each combine

_DEC_LO = np.array([-0.12940952255092145, 0.22414386804185735,
                    0.836516303737469, 0.48296291314469025], dtype=np.float64)
_C64 = np.cumsum(_DEC_LO)  # piecewise-constant segment values (scale-free)

_CACHE = {}


def _build_nc():
    import concourse.bass as bass
    import concourse.mybir as mybir
    from concourse.tile import TileContext

    F32 = mybir.dt.float32
    MULT = mybir.AluOpType.mult
    ADD = mybir.AluOpType.add
    RS = O - CS

    nc = bass.Bass("TRN2", target_bir_lowering=False, debug=False)
    x_d = nc.dram_tensor("x", [128, W], F32, kind="ExternalInput")
    out_d = nc.dram_tensor("out", [RPC, NS, OLEN], F32, kind="ExternalOutput")
    x_ap = x_d.ap()
    out_ap = out_d.ap()

    with TileContext(nc) as tc:
        with tc.tile_pool(name="pool", bufs=1) as pool:
            x_t = pool.tile([128, W], F32, tag="x", bufs=1, name="x_t")
            nc.sync.dma_start(out=x_t[:, :], in_=x_ap[:, :])

            def combine(j, src):
                s = 1 << j
                kv = [float(v)
                      for v in (_C64 * (2.0 ** -0.5) ** j).astype(np.float32)]
                r0 = float(_C64[0] / _C64[1])
                r1 = float(_C64[1] / _C64[2])
                r2 = float(_C64[2] / _C64[3])
                if j == 0:
                    a_o, b_o, c_o, d_o = H - 1, H, H + 1, H + 2
                    src = x_t
                else:
                    q = 3 * s // 2
                    a_o, b_o, c_o = H - q, H - q + s, H - q + 2 * s
                    d_o = H + q
                o_t = pool.tile([128, O], F32, tag="O", bufs=2, name=f"o_{j}")
                # left columns: DVE Horner chain, ACT final scale
                k1 = pool.tile([128, CS], F32, tag="K1", bufs=1, name=f"k1_{j}")
                k2 = pool.tile([128, CS], F32, tag="K2", bufs=1, name=f"k2_{j}")
                k3 = pool.tile([128, CS], F32, tag="K3", bufs=2, name=f"k3_{j}")
                nc.vector.scalar_tensor_tensor(
                    out=k1[:, :], in0=src[:, a_o:a_o + CS],
                    scalar=r0, in1=src[:, b_o:b_o + CS], op0=MULT, op1=ADD)
                nc.vector.scalar_tensor_tensor(
                    out=k2[:, :], in0=k1[:, :],
                    scalar=r1, in1=src[:, c_o:c_o + CS], op0=MULT, op1=ADD)
                nc.vector.scalar_tensor_tensor(
                    out=k3[:, :], in0=k2[:, :],
                    scalar=r2, in1=x_t[:, d_o:d_o + CS], op0=MULT, op1=ADD)
                nc.scalar.mul(o_t[:, 0:CS], k3[:, :], kv[3])
                # right columns: ACT scaled taps, GPSIMD pair adds
                m01 = pool.tile([128, 2 * RS], F32, tag="M01", bufs=1,
                                name=f"m01_{j}")
                m23 = pool.tile([128, 2 * RS], F32, tag="M23", bufs=1,
                                name=f"m23_{j}")
                nc.scalar.mul(m01[:, 0:RS], src[:, a_o + CS:a_o + O], kv[0])
                nc.scalar.mul(m01[:, RS:2 * RS], src[:, b_o + CS:b_o + O], kv[1])
                nc.scalar.mul(m23[:, 0:RS], src[:, c_o + CS:c_o + O], kv[2])
                nc.scalar.mul(m23[:, RS:2 * RS], x_t[:, d_o + CS:d_o + O], kv[3])
                t1 = pool.tile([128, RS], F32, tag="T1", bufs=1, name=f"t1_{j}")
                t2 = pool.tile([128, RS], F32, tag="T2", bufs=1, name=f"t2_{j}")
                nc.gpsimd.tensor_add(out=t1[:, :], in0=m01[:, 0:RS],
                                     in1=m01[:, RS:2 * RS])
                nc.gpsimd.tensor_add(out=t2[:, :], in0=m23[:, 0:RS],
                                     in1=m23[:, RS:2 * RS])
                nc.gpsimd.tensor_add(out=o_t[:, CS:O], in0=t1[:, :],
                                     in1=t2[:, :])
                # chunks 0-2 as one 96-partition DMA; chunk 3 separately
                # (its last column would spill past the 16383-long row)
                dst012 = out_ap[:, j, 0:3 * O].rearrange(
                    "r (c t) -> r c t", t=O).transpose([1, 0, 2])
                nc.sync.dma_start(out=dst012, in_=o_t[0:96, 0:O])
                nc.sync.dma_start(out=out_ap[:, j, 3 * O:OLEN],
                                  in_=o_t[96:128, 0:O - 1])

            combine(0, None)
            s_prev = x_t
            for k in range(1, J + 1):
                h = 1 << (k - 1)
                ln = W - (1 << k) + 1
                s_k = pool.tile([128, W], F32, tag="S", bufs=3, name=f"s_{k}")
                nc.vector.tensor_add(
                    out=s_k[:, 0:ln], in0=s_prev[:, 0:ln],
                    in1=s_prev[:, h:h + ln])
                combine(k, s_k)
                s_prev = s_k

    _split_sync_waits(nc, mybir)
    return nc


def _split_sync_waits(nc, mybir):
    """Walrus codegen packs sync waits into instruction encodings with very
    few slots (1 for STT/TT formats). Move excess waits onto standalone
    EventSemaphore instructions inserted just before the owner."""
    split_types = ("InstTensorScalarPtr", "InstTensorTensor", "InstActivation",
                   "InstDMACopy", "InstMemset", "InstTensorCopy", "InstDrain",
                   "InstMatmult", "InstLdweights", "InstTensorReduce",
                   "InstTensorScalar")
    n = 0
    for blk in nc.m.functions[0].blocks:
        new = []
        changed = False
        for ins in blk.instructions:
            si = ins.sync_info
            if (si is not None and len(si.on_wait) > 1
                    and type(ins).__name__ in split_types):
                waits = list(si.on_wait)
                for w in waits[:-1]:
                    ev = mybir.InstEventSemaphore(
                        name=f"waitsplit_{n}", ins=[], outs=[],
                        engine=ins.engine,
                        sync_info=mybir.SyncInfo(on_wait=[w], on_update=[]))
                    new.append(ev)
                    n += 1
                ins.sync_info = mybir.SyncInfo(on_wait=[waits[-1]],
                                               on_update=list(si.on_update))
                changed = True
            new.append(ins)
        if changed:
            blk.instructions = new


def _get_nc():
    if "nc" not in _CACHE:
        _CACHE["nc"] = _build_nc()
    return _CACHE["nc"]


def _prep_shard(shard):
    """(32, 16384) -> halo-replicated (128, 4864): partition c*32+r holds
    x[r, c*4096-384 : c*4096+4480] with zeros outside the sequence."""
    xpad = np.zeros((RPC, S + 2 * H), dtype=np.float32)
    xpad[:, H:H + S] = shard
    out = np.empty((4, RPC, W), dtype=np.float32)
    for c in range(4):
        out[c] = xpad[:, c * O:c * O + W]
    return np.ascontiguousarray(out.reshape(128, W))


def _run(x, trace=False):
    from concourse.bass_utils import run_bass_kernel_spmd

    x = np.asarray(x)
    xs = np.ascontiguousarray(x.reshape(B, S).astype(np.float32, copy=False))
    in_maps = [{"x": _prep_shard(xs[i * RPC:(i + 1) * RPC])}
               for i in range(NCORES)]
    nc = _get_nc()
    res = run_bass_kernel_spmd(nc, in_maps, core_ids=list(range(NCORES)),
                               trace=trace)
    out = np.concatenate([res.results[i]["out"] for i in range(NCORES)], axis=0)
    return out, res


def kernel(x):
    out, _ = _run(x, trace=False)
    return out


# revision 12
# speedup vs baseline: 54629.3993x; 54629.3993x over previous
"""A-trous wavelet decomposition (db2, J=8) Trainium2 Bass kernel.

Math: each scale-j filter h_j (length 3*2^j + 1) is the cumsum of the
dilated db2 dec_lo, i.e. piecewise constant with 4 segments:
  h_j = [c0]*s + [c1]*s + [c2]*s + [c3],  s = 2^j,  c_i = cumsum(dec_lo)_i * 2^(-j/2)
So conv(x, h_j) = c0*S_j[t-3s/2] + c1*S_j[t-s/2] + c2*S_j[t+s/2] + c3*x[t+3s/2]
with S_j = sliding window sum of length 2^j, built by recursive doubling:
  S_k[i] = S_{k-1}[i] + S_{k-1}[i + 2^(k-1)]

Sharding: pure data parallel over batch (256 = 8 cores x 32 rows).
Per-core layout: 128 partitions = 4 sequence chunks x 32 rows, each
partition holds a 4096-column chunk plus 384-column halos (max tap reach
at j=8 is +-384). The halo-replicated [128, 4864] layout is prepared on
the host so the device input is a single contiguous DMA.

Engine balance (cols of each 4096-wide output row):
  - left [0, CS): Vector engine runs the fused Horner chain
    (scalar_tensor_tensor: out = in0*r + in1), Scalar engine applies the
    final scale multiply.
  - right [CS, 4096): Scalar engine computes the 4 scaled taps, GPSIMD
    pair-adds them.
  - window-sum pyramid: Vector engine.
This splits the add-bound work across DVE+GPSIMD and the multiply work
onto ACT, with all three engines ~equally busy.

Walrus codegen only has 1-2 sync-wait slots per compute-instruction
encoding, so a post-pass moves excess Tile-generated waits onto
standalone EventSemaphore instructions.
"""

import numpy as np

B, S = 256, 16384
NCORES = 8
RPC = B // NCORES          # rows per core = 32
J = 8
NS = J + 1                 # 9 scales
OLEN = S - 1               # 16383 output length
H = 3 * (1 << (J - 1))     # 384 halo
O = 4096                   # output columns per partition-chunk
W = O + 2 * H              # 4864 columns per partition incl. halos
CS = 2208                  # DVE/(ACT+GPSIMD) column split of each combine

_DEC_LO = np.array([-0.12940952255092145, 0.22414386804185735,
                    0.836516303737469, 0.48296291314469025], dtype=np.float64)
_C64 = np.cumsum(_DEC_LO)  # piecewise-constant segment values (scale-free)

_CACHE = {}


def _build_nc():
    import concourse.bass as bass
    import concourse.mybir as mybir
    from concourse.tile import TileContext

    F32 = mybir.dt.float32
    MULT = mybir.AluOpType.mult
    ADD = mybir.AluOpType.add
    RS = O - CS

    nc = bass.Bass("TRN2", target_bir_lowering=False, debug=False)
    x_d = nc.dram_tensor("x", [128, W], F32, kind="ExternalInput")
    out_d = nc.dram_tensor("out", [RPC, NS, OLEN], F32, kind="ExternalOutput")
    x_ap = x_d.ap()
    out_ap = out_d.ap()

    with TileContext(nc) as tc:
        with tc.tile_pool(name="pool", bufs=1) as pool:
            x_t = pool.tile([128, W], F32, tag="x", bufs=1, name="x_t")
            nc.sync.dma_start(out=x_t[:, :], in_=x_ap[:, :])

            def combine(j, src):
                s = 1 << j
                kv = [float(v)
                      for v in (_C64 * (2.0 ** -0.5) ** j).astype(np.float32)]
                r0 = float(_C64[0] / _C64[1])
                r1 = float(_C64[1] / _C64[2])
                r2 = float(_C64[2] / _C64[3])
                if j == 0:
                    a_o, b_o, c_o, d_o = H - 1, H, H + 1, H + 2
                    src = x_t
                else:
                    q = 3 * s // 2
                    a_o, b_o, c_o = H - q, H - q + s, H - q + 2 * s
                    d_o = H + q
                o_t = pool.tile([128, O], F32, tag="O", bufs=2, name=f"o_{j}")
                # left columns: DVE Horner chain, ACT final scale
                k1 = pool.tile([128, CS], F32, tag="K1", bufs=1, name=f"k1_{j}")
                k2 = pool.tile([128, CS], F32, tag="K2", bufs=1, name=f"k2_{j}")
                k3 = pool.tile([128, CS], F32, tag="K3", bufs=2, name=f"k3_{j}")
                nc.vector.scalar_tensor_tensor(
                    out=k1[:, :], in0=src[:, a_o:a_o + CS],
                    scalar=r0, in1=src[:, b_o:b_o + CS], op0=MULT, op1=ADD)
                nc.vector.scalar_tensor_tensor(
                    out=k2[:, :], in0=k1[:, :],
                    scalar=r1, in1=src[:, c_o:c_o + CS], op0=MULT, op1=ADD)
                nc.vector.scalar_tensor_tensor(
                    out=k3[:, :], in0=k2[:, :],
                    scalar=r2, in1=x_t[:, d_o:d_o + CS], op0=MULT, op1=ADD)
                nc.scalar.mul(o_t[:, 0:CS], k3[:, :], kv[3])
                # right columns: ACT scaled taps, GPSIMD pair adds
                m01 = pool.tile([128, 2 * RS], F32, tag="M01", bufs=1,
                                name=f"m01_{j}")
                m23 = pool.tile([128, 2 * RS], F32, tag="M23", bufs=1,
                                name=f"m23_{j}")
                nc.scalar.mul(m01[:, 0:RS], src[:, a_o + CS:a_o + O], kv[0])
                nc.scalar.mul(m01[:, RS:2 * RS], src[:, b_o + CS:b_o + O], kv[1])
                nc.scalar.mul(m23[:, 0:RS], src[:, c_o + CS:c_o + O], kv[2])
                nc.scalar.mul(m23[:, RS:2 * RS], x_t[:, d_o + CS:d_o + O], kv[3])
                t1 = pool.tile([128, RS], F32, tag="T1", bufs=1, name=f"t1_{j}")
                t2 = pool.tile([128, RS], F32, tag="T2", bufs=1, name=f"t2_{j}")
                nc.gpsimd.tensor_add(out=t1[:, :], in0=m01[:, 0:RS],
                                     in1=m01[:, RS:2 * RS])
                nc.gpsimd.tensor_add(out=t2[:, :], in0=m23[:, 0:RS],
                                     in1=m23[:, RS:2 * RS])
                nc.gpsimd.tensor_add(out=o_t[:, CS:O], in0=t1[:, :],
                                     in1=t2[:, :])
                # chunks 0-2 as one 96-partition DMA; chunk 3 separately
                # (its last column would spill past the 16383-long row)
                dst012 = out_ap[:, j, 0:3 * O].rearrange(
                    "r (c t) -> r c t", t=O).transpose([1, 0, 2])
                nc.sync.dma_start(out=dst012, in_=o_t[0:96, 0:O])
                nc.sync.dma_start(out=out_ap[:, j, 3 * O:OLEN],
                                  in_=o_t[96:128, 0:O - 1])

            combine(0, None)
            s_prev = x_t
            for k in range(1, J + 1):
                h = 1 << (k - 1)
                ln = W - (1 << k) + 1
                s_k = pool.tile([128, W], F32, tag="S", bufs=3, name=f"s_{k}")
                nc.vector.tensor_add(
                    out=s_k[:, 0:ln], in0=s_prev[:, 0:ln],
                    in1=s_prev[:, h:h + ln])
                combine(k, s_k)
                s_prev = s_k

    _split_sync_waits(nc, mybir)
    return nc


def _split_sync_waits(nc, mybir):
    """Walrus codegen packs sync waits into instruction encodings with very
    few slots (1 for STT/TT formats). Move excess waits onto standalone
    EventSemaphore instructions inserted just before the owner."""
    split_types = ("InstTensorScalarPtr", "InstTensorTensor", "InstActivation",
                   "InstDMACopy", "InstMemset", "InstTensorCopy", "InstDrain",
                   "InstMatmult", "InstLdweights", "InstTensorReduce",
                   "InstTensorScalar")
    n = 0
    for blk in nc.m.functions[0].blocks:
        new = []
        changed = False
        for ins in blk.instructions:
            si = ins.sync_info
            if (si is not None and len(si.on_wait) > 1
                    and type(ins).__name__ in split_types):
                waits = list(si.on_wait)
                for w in waits[:-1]:
                    ev = mybir.InstEventSemaphore(
                        name=f"waitsplit_{n}", ins=[], outs=[],
                        engine=ins.engine,
                        sync_info=mybir.SyncInfo(on_wait=[w], on_update=[]))
                    new.append(ev)
                    n += 1
                ins.sync_info = mybir.SyncInfo(on_wait=[waits[-1]],
                                               on_update=list(si.on_update))
                changed = True
            new.append(ins)
        if changed:
            blk.instructions = new


def _get_nc():
    if "nc" not in _CACHE:
        _CACHE["nc"] = _build_nc()
    return _CACHE["nc"]


def _prep_shard(shard):
    """(32, 16384) -> halo-replicated (128, 4864): partition c*32+r holds
    x[r, c*4096-384 : c*4096+4480] with zeros outside the sequence."""
    xpad = np.zeros((RPC, S + 2 * H), dtype=np.float32)
    xpad[:, H:H + S] = shard
    out = np.empty((4, RPC, W), dtype=np.float32)
    for c in range(4):
        out[c] = xpad[:, c * O:c * O + W]
    return np.ascontiguousarray(out.reshape(128, W))


def _run(x, trace=False):
    from concourse.bass_utils import run_bass_kernel_spmd

    x = np.asarray(x)
    xs = np.ascontiguousarray(x.reshape(B, S).astype(np.float32, copy=False))
    in_maps = [{"x": _prep_shard(xs[i * RPC:(i + 1) * RPC])}
               for i in range(NCORES)]
    nc = _get_nc()
    res = run_bass_kernel_spmd(nc, in_maps, core_ids=list(range(NCORES)),
                               trace=trace)
    out = np.concatenate([res.results[i]["out"] for i in range(NCORES)], axis=0)
    return out, res


def kernel(x):
    out, _ = _run(x, trace=False)
    return out


# revision 15
# speedup vs baseline: 55100.1450x; 1.0086x over previous
"""A-trous wavelet decomposition (db2, J=8) Trainium2 Bass kernel.

Math: each scale-j filter h_j (length 3*2^j + 1) is the cumsum of the
dilated db2 dec_lo, i.e. piecewise constant with 4 segments:
  h_j = [c0]*s + [c1]*s + [c2]*s + [c3],  s = 2^j,  c_i = cumsum(dec_lo)_i * 2^(-j/2)
So conv(x, h_j) = c0*S_j[t-3s/2] + c1*S_j[t-s/2] + c2*S_j[t+s/2] + c3*x[t+3s/2]
with S_j = sliding window sum of length 2^j, built by recursive doubling:
  S_k[i] = S_{k-1}[i] + S_{k-1}[i + 2^(k-1)]

Sharding: pure data parallel over batch (256 = 8 cores x 32 rows).
Per-core layout: 128 partitions = 4 sequence chunks x 32 rows, each
partition holds a 4096-column chunk plus 384-column halos (max tap reach
at j=8 is +-384). The halo-replicated [128, 4864] layout is prepared on
the host so the device input is a single contiguous DMA.

Engine balance (cols of each 4096-wide output row):
  - left [0, CS): Vector engine runs the fused Horner chain
    (scalar_tensor_tensor: out = in0*r + in1), Scalar engine applies the
    final scale multiply.
  - right [CS, 4096): Scalar engine computes the 4 scaled taps, GPSIMD
    pair-adds them.
  - window-sum pyramid: Vector engine.
This splits the add-bound work across DVE+GPSIMD and the multiply work
onto ACT, with all three engines ~equally busy.

Walrus codegen only has 1-2 sync-wait slots per compute-instruction
encoding, so a post-pass moves excess Tile-generated waits onto
standalone EventSemaphore instructions.
"""

import numpy as np

B, S = 256, 16384
NCORES = 8
RPC = B // NCORES          # rows per core = 32
J = 8
NS = J + 1                 # 9 scales
OLEN = S - 1               # 16383 output length
H = 3 * (1 << (J - 1))     # 384 halo
O = 4096                   # output columns per partition-chunk
W = O + 2 * H              # 4864 columns per partition incl. halos
CS = 2208                  # DVE/(ACT+GPSIMD) column split of each combine
ISPL = 2600                # input-load column split (overlap load/compute)

_DEC_LO = np.array([-0.12940952255092145, 0.22414386804185735,
                    0.836516303737469, 0.48296291314469025], dtype=np.float64)
_C64 = np.cumsum(_DEC_LO)  # piecewise-constant segment values (scale-free)

_CACHE = {}


def _build_nc():
    import concourse.bass as bass
    import concourse.mybir as mybir
    from concourse.tile import TileContext

    F32 = mybir.dt.float32
    MULT = mybir.AluOpType.mult
    ADD = mybir.AluOpType.add
    RS = O - CS

    nc = bass.Bass("TRN2", target_bir_lowering=False, debug=False)
    x_d = nc.dram_tensor("x", [128, W], F32, kind="ExternalInput")
    out_d = nc.dram_tensor("out", [RPC, NS, OLEN], F32, kind="ExternalOutput")
    x_ap = x_d.ap()
    out_ap = out_d.ap()

    with TileContext(nc) as tc:
        with tc.tile_pool(name="pool", bufs=1) as pool:
            x_t = pool.tile([128, W], F32, tag="x", bufs=1, name="x_t")
            # split the load so left-column compute can start ~3us earlier
            nc.sync.dma_start(out=x_t[:, 0:ISPL], in_=x_ap[:, 0:ISPL])
            nc.sync.dma_start(out=x_t[:, ISPL:W], in_=x_ap[:, ISPL:W])

            def combine(j, src):
                s = 1 << j
                kv = [float(v)
                      for v in (_C64 * (2.0 ** -0.5) ** j).astype(np.float32)]
                r0 = float(_C64[0] / _C64[1])
                r1 = float(_C64[1] / _C64[2])
                r2 = float(_C64[2] / _C64[3])
                if j == 0:
                    a_o, b_o, c_o, d_o = H - 1, H, H + 1, H + 2
                    src = x_t
                else:
                    q = 3 * s // 2
                    a_o, b_o, c_o = H - q, H - q + s, H - q + 2 * s
                    d_o = H + q
                o_t = pool.tile([128, O], F32, tag="O", bufs=2, name=f"o_{j}")
                # left columns: DVE Horner chain, ACT final scale
                k1 = pool.tile([128, CS], F32, tag="K1", bufs=1, name=f"k1_{j}")
                k2 = pool.tile([128, CS], F32, tag="K2", bufs=1, name=f"k2_{j}")
                k3 = pool.tile([128, CS], F32, tag="K3", bufs=2, name=f"k3_{j}")
                nc.vector.scalar_tensor_tensor(
                    out=k1[:, :], in0=src[:, a_o:a_o + CS],
                    scalar=r0, in1=src[:, b_o:b_o + CS], op0=MULT, op1=ADD)
                nc.vector.scalar_tensor_tensor(
                    out=k2[:, :], in0=k1[:, :],
                    scalar=r1, in1=src[:, c_o:c_o + CS], op0=MULT, op1=ADD)
                nc.vector.scalar_tensor_tensor(
                    out=k3[:, :], in0=k2[:, :],
                    scalar=r2, in1=x_t[:, d_o:d_o + CS], op0=MULT, op1=ADD)
                nc.scalar.mul(o_t[:, 0:CS], k3[:, :], kv[3])
                # right columns: ACT scaled taps, GPSIMD pair adds
                m01 = pool.tile([128, 2 * RS], F32, tag="M01", bufs=1,
                                name=f"m01_{j}")
                m23 = pool.tile([128, 2 * RS], F32, tag="M23", bufs=1,
                                name=f"m23_{j}")
                nc.scalar.mul(m01[:, 0:RS], src[:, a_o + CS:a_o + O], kv[0])
                nc.scalar.mul(m01[:, RS:2 * RS], src[:, b_o + CS:b_o + O], kv[1])
                nc.scalar.mul(m23[:, 0:RS], src[:, c_o + CS:c_o + O], kv[2])
                nc.scalar.mul(m23[:, RS:2 * RS], x_t[:, d_o + CS:d_o + O], kv[3])
                t1 = pool.tile([128, RS], F32, tag="T1", bufs=1, name=f"t1_{j}")
                t2 = pool.tile([128, RS], F32, tag="T2", bufs=1, name=f"t2_{j}")
                nc.gpsimd.tensor_add(out=t1[:, :], in0=m01[:, 0:RS],
                                     in1=m01[:, RS:2 * RS])
                nc.gpsimd.tensor_add(out=t2[:, :], in0=m23[:, 0:RS],
                                     in1=m23[:, RS:2 * RS])
                nc.gpsimd.tensor_add(out=o_t[:, CS:O], in0=t1[:, :],
                                     in1=t2[:, :])
                # chunks 0-2 as one 96-partition DMA; chunk 3 separately
                # (its last column would spill past the 16383-long row)
                dst012 = out_ap[:, j, 0:3 * O].rearrange(
                    "r (c t) -> r c t", t=O).transpose([1, 0, 2])
                nc.sync.dma_start(out=dst012, in_=o_t[0:96, 0:O])
                nc.sync.dma_start(out=out_ap[:, j, 3 * O:OLEN],
                                  in_=o_t[96:128, 0:O - 1])

            combine(0, None)
            s_prev = x_t
            for k in range(1, J + 1):
                h = 1 << (k - 1)
                ln = W - (1 << k) + 1
                s_k = pool.tile([128, W], F32, tag="S", bufs=3, name=f"s_{k}")
                if k == 1:
                    # split level 1 so its left part only needs the first
                    # input-load half
                    e1 = ISPL - 1
                    nc.vector.tensor_add(
                        out=s_k[:, 0:e1], in0=s_prev[:, 0:e1],
                        in1=s_prev[:, h:h + e1])
                    nc.vector.tensor_add(
                        out=s_k[:, e1:ln], in0=s_prev[:, e1:ln],
                        in1=s_prev[:, h + e1:h + ln])
                else:
                    nc.vector.tensor_add(
                        out=s_k[:, 0:ln], in0=s_prev[:, 0:ln],
                        in1=s_prev[:, h:h + ln])
                combine(k, s_k)
                s_prev = s_k

    _split_sync_waits(nc, mybir)
    return nc


def _split_sync_waits(nc, mybir):
    """Walrus codegen packs sync waits into instruction encodings with very
    few slots (1 for STT/TT formats). Move excess waits onto standalone
    EventSemaphore instructions inserted just before the owner."""
    split_types = ("InstTensorScalarPtr", "InstTensorTensor", "InstActivation",
                   "InstDMACopy", "InstMemset", "InstTensorCopy", "InstDrain",
                   "InstMatmult", "InstLdweights", "InstTensorReduce",
                   "InstTensorScalar")
    n = 0
    for blk in nc.m.functions[0].blocks:
        new = []
        changed = False
        for ins in blk.instructions:
            si = ins.sync_info
            if (si is not None and len(si.on_wait) > 1
                    and type(ins).__name__ in split_types):
                waits = list(si.on_wait)
                for w in waits[:-1]:
                    ev = mybir.InstEventSemaphore(
                        name=f"waitsplit_{n}", ins=[], outs=[],
                        engine=ins.engine,
                        sync_info=mybir.SyncInfo(on_wait=[w], on_update=[]))
                    new.append(ev)
                    n += 1
                ins.sync_info = mybir.SyncInfo(on_wait=[waits[-1]],
                                               on_update=list(si.on_update))
                changed = True
            new.append(ins)
        if changed:
            blk.instructions = new


def _get_nc():
    if "nc" not in _CACHE:
        _CACHE["nc"] = _build_nc()
    return _CACHE["nc"]


def _prep_shard(shard):
    """(32, 16384) -> halo-replicated (128, 4864): partition c*32+r holds
    x[r, c*4096-384 : c*4096+4480] with zeros outside the sequence."""
    xpad = np.zeros((RPC, S + 2 * H), dtype=np.float32)
    xpad[:, H:H + S] = shard
    out = np.empty((4, RPC, W), dtype=np.float32)
    for c in range(4):
        out[c] = xpad[:, c * O:c * O + W]
    return np.ascontiguousarray(out.reshape(128, W))


def _run(x, trace=False):
    from concourse.bass_utils import run_bass_kernel_spmd

    x = np.asarray(x)
    xs = np.ascontiguousarray(x.reshape(B, S).astype(np.float32, copy=False))
    in_maps = [{"x": _prep_shard(xs[i * RPC:(i + 1) * RPC])}
               for i in range(NCORES)]
    nc = _get_nc()
    res = run_bass_kernel_spmd(nc, in_maps, core_ids=list(range(NCORES)),
                               trace=trace)
    out = np.concatenate([res.results[i]["out"] for i in range(NCORES)], axis=0)
    return out, res


def kernel(x):
    out, _ = _run(x, trace=False)
    return out


# revision 16
# speedup vs baseline: 56323.5627x; 1.0222x over previous
"""A-trous wavelet decomposition (db2, J=8) Trainium2 Bass kernel.

Math: each scale-j filter h_j (length 3*2^j + 1) is the cumsum of the
dilated db2 dec_lo, i.e. piecewise constant with 4 segments:
  h_j = [c0]*s + [c1]*s + [c2]*s + [c3],  s = 2^j,  c_i = cumsum(dec_lo)_i * 2^(-j/2)
So conv(x, h_j) = c0*S_j[t-3s/2] + c1*S_j[t-s/2] + c2*S_j[t+s/2] + c3*x[t+3s/2]
with S_j = sliding window sum of length 2^j, built by recursive doubling:
  S_k[i] = S_{k-1}[i] + S_{k-1}[i + 2^(k-1)]

Sharding: pure data parallel over batch (256 = 8 cores x 32 rows).
Per-core layout: 128 partitions = 4 sequence chunks x 32 rows, each
partition holds a 4096-column chunk plus 384-column halos (max tap reach
at j=8 is +-384). The halo-replicated [128, 4864] layout is prepared on
the host so the device input is a single contiguous DMA.

Engine balance (cols of each 4096-wide output row):
  - left [0, CS): Vector engine runs the fused Horner chain
    (scalar_tensor_tensor: out = in0*r + in1), Scalar engine applies the
    final scale multiply.
  - right [CS, 4096): Scalar engine computes the 4 scaled taps, GPSIMD
    pair-adds them.
  - window-sum pyramid: Vector engine.
This splits the add-bound work across DVE+GPSIMD and the multiply work
onto ACT, with all three engines ~equally busy.

Walrus codegen only has 1-2 sync-wait slots per compute-instruction
encoding, so a post-pass moves excess Tile-generated waits onto
standalone EventSemaphore instructions.
"""

import numpy as np

B, S = 256, 16384
NCORES = 8
RPC = B // NCORES          # rows per core = 32
J = 8
NS = J + 1                 # 9 scales
OLEN = S - 1               # 16383 output length
H = 3 * (1 << (J - 1))     # 384 halo
O = 4096                   # output columns per partition-chunk
W = O + 2 * H              # 4864 columns per partition incl. halos
# DVE/(ACT+GPSIMD) column split of each combine; the last two scales are
# more DVE-heavy so the GPSIMD/store tail after the final pyramid level is
# shorter.
CS_LIST = [2208] * 7 + [2400, 2700]
ISPL = 2600                # input-load column split (overlap load/compute)

_DEC_LO = np.array([-0.12940952255092145, 0.22414386804185735,
                    0.836516303737469, 0.48296291314469025], dtype=np.float64)
_C64 = np.cumsum(_DEC_LO)  # piecewise-constant segment values (scale-free)

_CACHE = {}


def _build_nc():
    import concourse.bass as bass
    import concourse.mybir as mybir
    from concourse.tile import TileContext

    F32 = mybir.dt.float32
    MULT = mybir.AluOpType.mult
    ADD = mybir.AluOpType.add

    nc = bass.Bass("TRN2", target_bir_lowering=False, debug=False)
    x_d = nc.dram_tensor("x", [128, W], F32, kind="ExternalInput")
    out_d = nc.dram_tensor("out", [RPC, NS, OLEN], F32, kind="ExternalOutput")
    x_ap = x_d.ap()
    out_ap = out_d.ap()

    with TileContext(nc) as tc:
        with tc.tile_pool(name="pool", bufs=1) as pool:
            x_t = pool.tile([128, W], F32, tag="x", bufs=1, name="x_t")
            # split the load so left-column compute can start ~3us earlier
            nc.sync.dma_start(out=x_t[:, 0:ISPL], in_=x_ap[:, 0:ISPL])
            nc.sync.dma_start(out=x_t[:, ISPL:W], in_=x_ap[:, ISPL:W])

            def combine(j, src):
                CS = CS_LIST[j]
                RS = O - CS
                s = 1 << j
                kv = [float(v)
                      for v in (_C64 * (2.0 ** -0.5) ** j).astype(np.float32)]
                r0 = float(_C64[0] / _C64[1])
                r1 = float(_C64[1] / _C64[2])
                r2 = float(_C64[2] / _C64[3])
                if j == 0:
                    a_o, b_o, c_o, d_o = H - 1, H, H + 1, H + 2
                    src = x_t
                else:
                    q = 3 * s // 2
                    a_o, b_o, c_o = H - q, H - q + s, H - q + 2 * s
                    d_o = H + q
                o_t = pool.tile([128, O], F32, tag="O", bufs=2, name=f"o_{j}")
                # left columns: DVE Horner chain, ACT final scale
                k1 = pool.tile([128, CS], F32, tag="K1", bufs=1, name=f"k1_{j}")
                k2 = pool.tile([128, CS], F32, tag="K2", bufs=1, name=f"k2_{j}")
                k3 = pool.tile([128, CS], F32, tag="K3", bufs=2, name=f"k3_{j}")
                nc.vector.scalar_tensor_tensor(
                    out=k1[:, :], in0=src[:, a_o:a_o + CS],
                    scalar=r0, in1=src[:, b_o:b_o + CS], op0=MULT, op1=ADD)
                nc.vector.scalar_tensor_tensor(
                    out=k2[:, :], in0=k1[:, :],
                    scalar=r1, in1=src[:, c_o:c_o + CS], op0=MULT, op1=ADD)
                nc.vector.scalar_tensor_tensor(
                    out=k3[:, :], in0=k2[:, :],
                    scalar=r2, in1=x_t[:, d_o:d_o + CS], op0=MULT, op1=ADD)
                nc.scalar.mul(o_t[:, 0:CS], k3[:, :], kv[3])
                # right columns: ACT scaled taps, GPSIMD pair adds
                m01 = pool.tile([128, 2 * RS], F32, tag="M01", bufs=1,
                                name=f"m01_{j}")
                m23 = pool.tile([128, 2 * RS], F32, tag="M23", bufs=1,
                                name=f"m23_{j}")
                nc.scalar.mul(m01[:, 0:RS], src[:, a_o + CS:a_o + O], kv[0])
                nc.scalar.mul(m01[:, RS:2 * RS], src[:, b_o + CS:b_o + O], kv[1])
                nc.scalar.mul(m23[:, 0:RS], src[:, c_o + CS:c_o + O], kv[2])
                nc.scalar.mul(m23[:, RS:2 * RS], x_t[:, d_o + CS:d_o + O], kv[3])
                t1 = pool.tile([128, RS], F32, tag="T1", bufs=1, name=f"t1_{j}")
                t2 = pool.tile([128, RS], F32, tag="T2", bufs=1, name=f"t2_{j}")
                nc.gpsimd.tensor_add(out=t1[:, :], in0=m01[:, 0:RS],
                                     in1=m01[:, RS:2 * RS])
                nc.gpsimd.tensor_add(out=t2[:, :], in0=m23[:, 0:RS],
                                     in1=m23[:, RS:2 * RS])
                nc.gpsimd.tensor_add(out=o_t[:, CS:O], in0=t1[:, :],
                                     in1=t2[:, :])
                # chunks 0-2 as one 96-partition DMA; chunk 3 separately
                # (its last column would spill past the 16383-long row)
                dst012 = out_ap[:, j, 0:3 * O].rearrange(
                    "r (c t) -> r c t", t=O).transpose([1, 0, 2])
                nc.sync.dma_start(out=dst012, in_=o_t[0:96, 0:O])
                nc.sync.dma_start(out=out_ap[:, j, 3 * O:OLEN],
                                  in_=o_t[96:128, 0:O - 1])

            combine(0, None)
            s_prev = x_t
            for k in range(1, J + 1):
                h = 1 << (k - 1)
                ln = W - (1 << k) + 1
                s_k = pool.tile([128, W], F32, tag="S", bufs=3, name=f"s_{k}")
                if k == 1:
                    # split level 1 so its left part only needs the first
                    # input-load half
                    e1 = ISPL - 1
                    nc.vector.tensor_add(
                        out=s_k[:, 0:e1], in0=s_prev[:, 0:e1],
                        in1=s_prev[:, h:h + e1])
                    nc.vector.tensor_add(
                        out=s_k[:, e1:ln], in0=s_prev[:, e1:ln],
                        in1=s_prev[:, h + e1:h + ln])
                else:
                    nc.vector.tensor_add(
                        out=s_k[:, 0:ln], in0=s_prev[:, 0:ln],
                        in1=s_prev[:, h:h + ln])
                combine(k, s_k)
                s_prev = s_k

    _split_sync_waits(nc, mybir)
    return nc


def _split_sync_waits(nc, mybir):
    """Walrus codegen packs sync waits into instruction encodings with very
    few slots (1 for STT/TT formats). Move excess waits onto standalone
    EventSemaphore instructions inserted just before the owner."""
    split_types = ("InstTensorScalarPtr", "InstTensorTensor", "InstActivation",
                   "InstDMACopy", "InstMemset", "InstTensorCopy", "InstDrain",
                   "InstMatmult", "InstLdweights", "InstTensorReduce",
                   "InstTensorScalar")
    n = 0
    for blk in nc.m.functions[0].blocks:
        new = []
        changed = False
        for ins in blk.instructions:
            si = ins.sync_info
            if (si is not None and len(si.on_wait) > 1
                    and type(ins).__name__ in split_types):
                waits = list(si.on_wait)
                for w in waits[:-1]:
                    ev = mybir.InstEventSemaphore(
                        name=f"waitsplit_{n}", ins=[], outs=[],
                        engine=ins.engine,
                        sync_info=mybir.SyncInfo(on_wait=[w], on_update=[]))
                    new.append(ev)
                    n += 1
                ins.sync_info = mybir.SyncInfo(on_wait=[waits[-1]],
                                               on_update=list(si.on_update))
                changed = True
            new.append(ins)
        if changed:
            blk.instructions = new


def _get_nc():
    if "nc" not in _CACHE:
        _CACHE["nc"] = _build_nc()
    return _CACHE["nc"]


def _prep_shard(shard):
    """(32, 16384) -> halo-replicated (128, 4864): partition c*32+r holds
    x[r, c*4096-384 : c*4096+4480] with zeros outside the sequence."""
    xpad = np.zeros((RPC, S + 2 * H), dtype=np.float32)
    xpad[:, H:H + S] = shard
    out = np.empty((4, RPC, W), dtype=np.float32)
    for c in range(4):
        out[c] = xpad[:, c * O:c * O + W]
    return np.ascontiguousarray(out.reshape(128, W))


def _run(x, trace=False):
    from concourse.bass_utils import run_bass_kernel_spmd

    x = np.asarray(x)
    xs = np.ascontiguousarray(x.reshape(B, S).astype(np.float32, copy=False))
    in_maps = [{"x": _prep_shard(xs[i * RPC:(i + 1) * RPC])}
               for i in range(NCORES)]
    nc = _get_nc()
    res = run_bass_kernel_spmd(nc, in_maps, core_ids=list(range(NCORES)),
                               trace=trace)
    out = np.concatenate([res.results[i]["out"] for i in range(NCORES)], axis=0)
    return out, res


def kernel(x):
    out, _ = _run(x, trace=False)
    return out


# revision 17
# speedup vs baseline: 57121.8066x; 1.0142x over previous
"""A-trous wavelet decomposition (db2, J=8) Trainium2 Bass kernel.

Math: each scale-j filter h_j (length 3*2^j + 1) is the cumsum of the
dilated db2 dec_lo, i.e. piecewise constant with 4 segments:
  h_j = [c0]*s + [c1]*s + [c2]*s + [c3],  s = 2^j,  c_i = cumsum(dec_lo)_i * 2^(-j/2)
So conv(x, h_j) = c0*S_j[t-3s/2] + c1*S_j[t-s/2] + c2*S_j[t+s/2] + c3*x[t+3s/2]
with S_j = sliding window sum of length 2^j, built by recursive doubling:
  S_k[i] = S_{k-1}[i] + S_{k-1}[i + 2^(k-1)]

Sharding: pure data parallel over batch (256 = 8 cores x 32 rows).
Per-core layout: 128 partitions = 4 sequence chunks x 32 rows, each
partition holds a 4096-column chunk plus 384-column halos (max tap reach
at j=8 is +-384). The halo-replicated [128, 4864] layout is prepared on
the host so the device input is a single contiguous DMA.

Engine balance (cols of each 4096-wide output row):
  - left [0, CS): Vector engine runs the fused Horner chain
    (scalar_tensor_tensor: out = in0*r + in1), Scalar engine applies the
    final scale multiply.
  - right [CS, 4096): Scalar engine computes the 4 scaled taps, GPSIMD
    pair-adds them.
  - window-sum pyramid: Vector engine.
This splits the add-bound work across DVE+GPSIMD and the multiply work
onto ACT, with all three engines ~equally busy.

Walrus codegen only has 1-2 sync-wait slots per compute-instruction
encoding, so a post-pass moves excess Tile-generated waits onto
standalone EventSemaphore instructions.
"""

import numpy as np

B, S = 256, 16384
NCORES = 8
RPC = B // NCORES          # rows per core = 32
J = 8
NS = J + 1                 # 9 scales
OLEN = S - 1               # 16383 output length
H = 3 * (1 << (J - 1))     # 384 halo
O = 4096                   # output columns per partition-chunk
W = O + 2 * H              # 4864 columns per partition incl. halos
# DVE/(ACT+GPSIMD) column split of each combine; the last two scales are
# more DVE-heavy so the GPSIMD/store tail after the final pyramid level is
# shorter.
CS_LIST = [2208] * 7 + [2400, 2900]
XC_SCALES = (7, 8)          # scales whose final multiply is pre-applied to
                            # the x-tap so the last STT writes output directly
ISPL = 2600                # input-load column split (overlap load/compute)

_DEC_LO = np.array([-0.12940952255092145, 0.22414386804185735,
                    0.836516303737469, 0.48296291314469025], dtype=np.float64)
_C64 = np.cumsum(_DEC_LO)  # piecewise-constant segment values (scale-free)

_CACHE = {}


def _build_nc():
    import concourse.bass as bass
    import concourse.mybir as mybir
    from concourse.tile import TileContext

    F32 = mybir.dt.float32
    MULT = mybir.AluOpType.mult
    ADD = mybir.AluOpType.add

    nc = bass.Bass("TRN2", target_bir_lowering=False, debug=False)
    x_d = nc.dram_tensor("x", [128, W], F32, kind="ExternalInput")
    out_d = nc.dram_tensor("out", [RPC, NS, OLEN], F32, kind="ExternalOutput")
    x_ap = x_d.ap()
    out_ap = out_d.ap()

    with TileContext(nc) as tc:
        with tc.tile_pool(name="pool", bufs=1) as pool:
            x_t = pool.tile([128, W], F32, tag="x", bufs=1, name="x_t")
            # split the load so left-column compute can start ~3us earlier
            nc.sync.dma_start(out=x_t[:, 0:ISPL], in_=x_ap[:, 0:ISPL])
            nc.sync.dma_start(out=x_t[:, ISPL:W], in_=x_ap[:, ISPL:W])

            def combine(j, src):
                CS = CS_LIST[j]
                RS = O - CS
                s = 1 << j
                kv = [float(v)
                      for v in (_C64 * (2.0 ** -0.5) ** j).astype(np.float32)]
                r0 = float(_C64[0] / _C64[1])
                r1 = float(_C64[1] / _C64[2])
                r2 = float(_C64[2] / _C64[3])
                if j == 0:
                    a_o, b_o, c_o, d_o = H - 1, H, H + 1, H + 2
                    src = x_t
                else:
                    q = 3 * s // 2
                    a_o, b_o, c_o = H - q, H - q + s, H - q + 2 * s
                    d_o = H + q
                o_t = pool.tile([128, O], F32, tag="O", bufs=2, name=f"o_{j}")
                # left columns: DVE Horner chain, ACT final scale
                k1 = pool.tile([128, CS], F32, tag="K1", bufs=1, name=f"k1_{j}")
                k2 = pool.tile([128, CS], F32, tag="K2", bufs=1, name=f"k2_{j}")
                if j not in XC_SCALES:
                    k3 = pool.tile([128, CS], F32, tag="K3", bufs=2,
                                   name=f"k3_{j}")
                nc.vector.scalar_tensor_tensor(
                    out=k1[:, :], in0=src[:, a_o:a_o + CS],
                    scalar=r0, in1=src[:, b_o:b_o + CS], op0=MULT, op1=ADD)
                nc.vector.scalar_tensor_tensor(
                    out=k2[:, :], in0=k1[:, :],
                    scalar=r1, in1=src[:, c_o:c_o + CS], op0=MULT, op1=ADD)
                if j in XC_SCALES:
                    # pre-scaled x-tap (ACT, depends only on x) lets the last
                    # STT write the final output, shortening the tail chain
                    xc = pool.tile([128, CS], F32, tag="XC", bufs=1,
                                   name=f"xc_{j}")
                    nc.scalar.mul(xc[:, :], x_t[:, d_o:d_o + CS], kv[3])
                    nc.vector.scalar_tensor_tensor(
                        out=o_t[:, 0:CS], in0=k2[:, :],
                        scalar=float(r2 * kv[3]), in1=xc[:, :],
                        op0=MULT, op1=ADD)
                else:
                    nc.vector.scalar_tensor_tensor(
                        out=k3[:, :], in0=k2[:, :],
                        scalar=r2, in1=x_t[:, d_o:d_o + CS], op0=MULT, op1=ADD)
                    nc.scalar.mul(o_t[:, 0:CS], k3[:, :], kv[3])
                # right columns: ACT scaled taps, GPSIMD pair adds
                m01 = pool.tile([128, 2 * RS], F32, tag="M01", bufs=1,
                                name=f"m01_{j}")
                m23 = pool.tile([128, 2 * RS], F32, tag="M23", bufs=1,
                                name=f"m23_{j}")
                nc.scalar.mul(m01[:, 0:RS], src[:, a_o + CS:a_o + O], kv[0])
                nc.scalar.mul(m01[:, RS:2 * RS], src[:, b_o + CS:b_o + O], kv[1])
                nc.scalar.mul(m23[:, 0:RS], src[:, c_o + CS:c_o + O], kv[2])
                nc.scalar.mul(m23[:, RS:2 * RS], x_t[:, d_o + CS:d_o + O], kv[3])
                t1 = pool.tile([128, RS], F32, tag="T1", bufs=1, name=f"t1_{j}")
                t2 = pool.tile([128, RS], F32, tag="T2", bufs=1, name=f"t2_{j}")
                nc.gpsimd.tensor_add(out=t1[:, :], in0=m01[:, 0:RS],
                                     in1=m01[:, RS:2 * RS])
                nc.gpsimd.tensor_add(out=t2[:, :], in0=m23[:, 0:RS],
                                     in1=m23[:, RS:2 * RS])
                nc.gpsimd.tensor_add(out=o_t[:, CS:O], in0=t1[:, :],
                                     in1=t2[:, :])
                # chunks 0-2 as one 96-partition DMA; chunk 3 separately
                # (its last column would spill past the 16383-long row)
                dst012 = out_ap[:, j, 0:3 * O].rearrange(
                    "r (c t) -> r c t", t=O).transpose([1, 0, 2])
                nc.sync.dma_start(out=dst012, in_=o_t[0:96, 0:O])
                nc.sync.dma_start(out=out_ap[:, j, 3 * O:OLEN],
                                  in_=o_t[96:128, 0:O - 1])

            combine(0, None)
            s_prev = x_t
            for k in range(1, J + 1):
                h = 1 << (k - 1)
                ln = W - (1 << k) + 1
                s_k = pool.tile([128, W], F32, tag="S", bufs=2, name=f"s_{k}")
                if k == 1:
                    # split level 1 so its left part only needs the first
                    # input-load half
                    e1 = ISPL - 1
                    nc.vector.tensor_add(
                        out=s_k[:, 0:e1], in0=s_prev[:, 0:e1],
                        in1=s_prev[:, h:h + e1])
                    nc.vector.tensor_add(
                        out=s_k[:, e1:ln], in0=s_prev[:, e1:ln],
                        in1=s_prev[:, h + e1:h + ln])
                else:
                    nc.vector.tensor_add(
                        out=s_k[:, 0:ln], in0=s_prev[:, 0:ln],
                        in1=s_prev[:, h:h + ln])
                combine(k, s_k)
                s_prev = s_k

    _split_sync_waits(nc, mybir)
    return nc


def _split_sync_waits(nc, mybir):
    """Walrus codegen packs sync waits into instruction encodings with very
    few slots (1 for STT/TT formats). Move excess waits onto standalone
    EventSemaphore instructions inserted just before the owner."""
    split_types = ("InstTensorScalarPtr", "InstTensorTensor", "InstActivation",
                   "InstDMACopy", "InstMemset", "InstTensorCopy", "InstDrain",
                   "InstMatmult", "InstLdweights", "InstTensorReduce",
                   "InstTensorScalar")
    n = 0
    for blk in nc.m.functions[0].blocks:
        new = []
        changed = False
        for ins in blk.instructions:
            si = ins.sync_info
            if (si is not None and len(si.on_wait) > 1
                    and type(ins).__name__ in split_types):
                waits = list(si.on_wait)
                for w in waits[:-1]:
                    ev = mybir.InstEventSemaphore(
                        name=f"waitsplit_{n}", ins=[], outs=[],
                        engine=ins.engine,
                        sync_info=mybir.SyncInfo(on_wait=[w], on_update=[]))
                    new.append(ev)
                    n += 1
                ins.sync_info = mybir.SyncInfo(on_wait=[waits[-1]],
                                               on_update=list(si.on_update))
                changed = True
            new.append(ins)
        if changed:
            blk.instructions = new


def _get_nc():
    if "nc" not in _CACHE:
        _CACHE["nc"] = _build_nc()
    return _CACHE["nc"]


def _prep_shard(shard):
    """(32, 16384) -> halo-replicated (128, 4864): partition c*32+r holds
    x[r, c*4096-384 : c*4096+4480] with zeros outside the sequence."""
    xpad = np.zeros((RPC, S + 2 * H), dtype=np.float32)
    xpad[:, H:H + S] = shard
    out = np.empty((4, RPC, W), dtype=np.float32)
    for c in range(4):
        out[c] = xpad[:, c * O:c * O + W]
    return np.ascontiguousarray(out.reshape(128, W))


def _run(x, trace=False):
    from concourse.bass_utils import run_bass_kernel_spmd

    x = np.asarray(x)
    xs = np.ascontiguousarray(x.reshape(B, S).astype(np.float32, copy=False))
    in_maps = [{"x": _prep_shard(xs[i * RPC:(i + 1) * RPC])}
               for i in range(NCORES)]
    nc = _get_nc()
    res = run_bass_kernel_spmd(nc, in_maps, core_ids=list(range(NCORES)),
                               trace=trace)
    out = np.concatenate([res.results[i]["out"] for i in range(NCORES)], axis=0)
    return out, res


def kernel(x):
    out, _ = _run(x, trace=False)
    return out
